# revision 21
# baseline (speedup 1.0000x reference)
"""2-layer GCN on 8 Trainium2 NeuronCores (Bass/Tile, SPMD).

softmax(A @ relu(A @ (X@W1) + b1) @ W2 + b2), N=50k nodes, E=800k edges.

Strategy (1D graph partition, token-packed gathers, 4 SWDGE queues):
- Nodes sharded 6250/core by global in-degree rank round-robin (rank k ->
  core k%8, local id k//8), padded to 6272 = 49*128 table rows per core.
  Local ids are descending-degree, so 128-node dst tiles have near-uniform
  degree and fixed per-tile slot counts waste little padding.
- Edges partitioned by dst owner. Layer-1 XW1 table is fp16 [50176, 64]
  -> 25088 pair-tokens of 256B; layer-2 HW2 stays fp32 [50176, 16] ->
  12544 4-row tokens of 256B (fp16 there would blow the softmax error
  budget). Token counts fit int16 gather indices.
- Per dst node, in-edges are deduped by (dst, token) into slots; each slot
  gathers one 256B token and a per-sub-row weight grid selects + weights
  rows during the DVE multiply (fp32 accumulate); a chunk-sum reduce
  completes the segment sum.
- dma_gather calls (1024 idxs each) round-robin over 4 SWDGE queues; the
  SWDGE queue descriptor rate (~11ns/desc/queue) is the wall.
- Slots are token-sorted per node, so each gather call's in_ap is sliced
  to [0, call_max_token]: early calls only depend on the first AllGather
  pieces and start while later pieces are still in flight.
- XW1 / HW2 tables are exchanged with 4 piece-wise AllGather collectives
  (piece-core-major table layout) overlapped with phase-1 / layer-1 compute.
"""

import sys

sys.path.insert(0, "/opt/trn_rl_repo")

import numpy as np

N = 50000
E = 800000
F = 512
HID = 64
NCLS = 16
NCORES = 8
P = 128
NPC = N // NCORES  # 6250
TILES = 49
TROWS = TILES * P  # 6272
CAP = 40  # max chunks per gather stage
NQ = 4  # SWDGE queues
SH1, SH2 = 1, 2  # token shifts: L1 fp16 pairs (256B), L2 fp32 4-row (256B)
S1, S2 = 1 << SH1, 1 << SH2
# AllGather pieces: tile groups whose collectives overlap phase-1/layer-1.
# AG1 pieces are fixed; AG2 pieces are chosen at preprocess time aligned to
# layer-1 stage boundaries, and layer-1 stages are processed in a rotated
# order so every AG2 piece except a tiny final one issues mid-stream.
PIECE_T1 = [(0, 13), (13, 25), (25, 37), (37, 49)]


def _piece_layout(piece_t):
    rows = [(t1 - t0) * P for (t0, t1) in piece_t]  # rows per core per piece
    cbs = [t0 * P for (t0, t1) in piece_t]          # row offset within a core
    pbs = [0]                                       # row offset in full table
    for pr in rows[:-1]:
        pbs.append(pbs[-1] + NCORES * pr)
    return rows, cbs, pbs


PIECE_ROWS1, CORE_BASE1, PIECE_BASE1 = _piece_layout(PIECE_T1)
# issue each piece's collective a couple of tiles after its data is complete
# so the gpsimd SEQ wait finds the shard writes done (it blocks later queue
# work otherwise)
AG1_AT = [15, 27, 39, TILES - 1]

_TRACE = False
LAST_EXEC_NS = None


def _build_grid(es_row, dl, ew, shift):
    """Slot grid for one core+layer: tokens = row>>shift, dedup (dst, token).

    Returns ig [P, ctot] int16 token grid, wsub [P, ctot*S] f32 sub-weights,
    cnt[TILES] per-tile chunk counts, stages list, off[TILES] col offsets.
    """
    S = 1 << shift
    tok = es_row >> shift
    sub = es_row & (S - 1)
    ntok_max = int(tok.max()) + 1 if len(tok) else 1
    order = np.lexsort((tok, dl))
    dls, toks, subs, ews = dl[order], tok[order], sub[order], ew[order]
    key = dls * ntok_max + toks
    new = np.r_[True, np.diff(key) != 0]
    slot_id = np.cumsum(new) - 1  # per (sorted) edge -> slot
    slot_dl = dls[new]
    slot_tok = toks[new]
    nslots = len(slot_tok)

    kd = np.bincount(slot_dl, minlength=NPC)  # unique-token count per node
    kd_pad = np.concatenate([kd, np.zeros(TROWS - NPC, np.int64)])
    cnt = kd_pad.reshape(TILES, P).max(1)  # per-tile chunk count

    # stages: consecutive tiles while chunk sum <= CAP
    stages = []
    off = np.zeros(TILES, dtype=np.int64)
    t0 = 0
    ctot = 0
    while t0 < TILES:
        t1, cs = t0, 0
        while t1 < TILES and cs + cnt[t1] <= CAP:
            off[t1] = ctot + cs
            cs += cnt[t1]
            t1 += 1
        stages.append((t0, t1, int(cs), ctot))
        ctot += cs
        t0 = t1
    ctot = int(ctot)

    # slot columns: per node, j-th slot at off[tile] + j
    starts = np.r_[0, np.cumsum(kd)[:-1]]
    j = np.arange(nslots) - starts[slot_dl]
    tl = slot_dl // P
    prow = slot_dl % P
    col = off[tl] + j

    ig = np.zeros((P, ctot), dtype=np.int16)
    ig[prow, col] = slot_tok.astype(np.int16)
    wsub = np.zeros((P, ctot * S), dtype=np.float32)
    e_col = col[slot_id]  # per sorted edge
    e_prow = prow[slot_id]
    np.add.at(wsub, (e_prow, e_col * S + subs), ews)
    return ig, wsub, cnt, stages, off, ctot


def _preprocess(src, dst, edge_weight):
    src = np.asarray(src).astype(np.int64).ravel()
    dst = np.asarray(dst).astype(np.int64).ravel()
    w = np.asarray(edge_weight).astype(np.float32).ravel()

    tdeg = np.bincount(dst, minlength=N)
    grank = np.empty(N, dtype=np.int64)
    grank[np.argsort(-tdeg, kind="stable")] = np.arange(N)
    owner_of = grank % NCORES
    lid_of = grank // NCORES  # descending-degree local order

    # full tables are piece-major then core-major (AllGather piece layout);
    # each layer has its own piece set, hence its own row numbering
    def row_map(cbs, rows, pbs):
        piece_of = np.searchsorted(np.array(cbs + [TROWS]), lid_of, side="right") - 1
        pb = np.array(pbs)[piece_of]
        prw = np.array(rows)[piece_of]
        cb = np.array(cbs)[piece_of]
        return pb + owner_of * prw + (lid_of - cb)

    row1_of = row_map(CORE_BASE1, PIECE_ROWS1, PIECE_BASE1)

    owner_dst = owner_of[dst]
    per_core = []
    for r in range(NCORES):
        m = owner_dst == r
        per_core.append((src[m], dst[m], w[m]))
    cores = []
    for es, ed, ew in per_core:
        dl = lid_of[ed]
        g1 = _build_grid(row1_of[es], dl, ew, SH1)  # layer 1: fp16 pair tokens
        cores.append([g1])

    # unify stage/offset structure across cores (max per-tile counts) so one
    # compiled program fits all cores
    def unify(idx, last_alone=False):
        cnt = np.stack([c[idx][2] for c in cores]).max(0)
        stages = []
        off = np.zeros(TILES, dtype=np.int64)
        lim = TILES - 1 if last_alone else TILES
        t0 = 0
        ctot = 0
        while t0 < lim:
            t1, cs = t0, 0
            while t1 < lim and cs + cnt[t1] <= CAP:
                off[t1] = ctot + cs
                cs += cnt[t1]
                t1 += 1
            stages.append((t0, t1, int(cs), ctot))
            ctot += cs
            t0 = t1
        if last_alone:
            off[TILES - 1] = ctot
            stages.append((TILES - 1, TILES, int(cnt[TILES - 1]), ctot))
            ctot += cnt[TILES - 1]
        return cnt, stages, off, int(ctot)

    # layer 1's last stage is tile 48 alone: AG2's final (single-tile) piece
    # only waits for one small tile's reduce, not a whole trailing stage
    uni1 = unify(0, last_alone=True)

    # AG2 is a single collective issued after the (tiny) final tile-48
    # stage: mid-stream collectives stall the gather queue ~15-20us each
    # while the gpsimd engine programs the CC rings, which costs more than
    # the serialized tail transfer they would hide.
    stages1 = uni1[1]
    piece_t2 = [(0, TILES)]
    rows2, cbs2, pbs2 = _piece_layout(piece_t2)
    order1 = stages1
    ag2_at = {TILES - 1: [0]}

    row2_of = row_map(cbs2, rows2, pbs2)
    for r, (es, ed, ew) in enumerate(per_core):
        dl = lid_of[ed]
        cores[r].append(_build_grid(row2_of[es], dl, ew, SH2))  # layer 2: fp32 4-row tokens
    uni2 = unify(1)

    # re-grid each core onto the unified layout
    def regrid(r, idx, uni, shift):
        S = 1 << shift
        ig_c, wsub_c, cnt_c, stages_c, off_c, ctot_c = cores[r][idx]
        cnt_u, stages_u, off_u, ctot_u = uni
        ig = np.zeros((P, ctot_u), dtype=np.int16)
        wsub = np.zeros((P, ctot_u * S), dtype=np.float32)
        for t in range(TILES):
            c = int(cnt_c[t])
            if c == 0:
                continue
            src_lo = int(off_c[t])
            dst_lo = int(off_u[t])
            ig[:, dst_lo:dst_lo + c] = ig_c[:, src_lo:src_lo + c]
            wsub[:, dst_lo * S:(dst_lo + c) * S] = wsub_c[:, src_lo * S:(src_lo + c) * S]
        return ig, wsub

    grids = []
    for r in range(NCORES):
        ig1, ws1 = regrid(r, 0, uni1, SH1)
        ig2, ws2 = regrid(r, 1, uni2, SH2)
        grids.append((ig1, ws1, ig2, ws2))

    # per-gather-call max token (over all cores): slots are token-sorted per
    # node, so early calls touch only early AllGather pieces -- slicing the
    # gather in_ap to [0, maxtok] lets them start before the later pieces
    # land.
    def call_maxtoks(gidx, uni):
        _, stages, _, _ = uni
        igs = np.stack([g[gidx] for g in grids])  # [NCORES, P, ctot]
        mts = []
        for (t0, t1, cs, c0) in stages:
            for o in range(0, cs, 8):
                n = min(8, cs - o)
                mts.append(int(igs[:, :, c0 + o : c0 + o + n].max()))
        return mts

    layout = dict(uni1=uni1, uni2=uni2, owner=owner_of, lid=lid_of,
                  mt1=call_maxtoks(0, uni1), mt2=call_maxtoks(2, uni2),
                  order1=order1, ag2_at=ag2_at,
                  ag2=(rows2, cbs2, pbs2))
    return layout, grids


def _wrap_idx(ig):
    """[128, C] token grid -> dma_gather wrapped idx array [128, C*8] int16."""
    seq = ig.T.reshape(-1)  # position q = c*128 + p
    cols = seq.shape[0] // 16
    seqm = seq.reshape(cols, 16).T  # [16, cols]
    return np.tile(seqm, (8, 1)).astype(np.int16)  # [128, cols]


def _build(layout):
    import concourse.bacc as bacc
    import concourse.tile as tile
    import concourse.mybir as mybir
    from concourse.masks import make_identity

    cnt1, stages1, off1, ctot1 = layout["uni1"]
    cnt2, stages2, off2, ctot2 = layout["uni2"]
    mt1, mt2 = layout["mt1"], layout["mt2"]
    order1, ag2_at = layout["order1"], layout["ag2_at"]
    PIECE_ROWS2, CORE_BASE2, PIECE_BASE2 = layout["ag2"]
    fp32 = mybir.dt.float32
    fp16 = mybir.dt.float16

    nc = bacc.Bacc(
        "TRN2", target_bir_lowering=False, debug=False, num_devices=NCORES,
        num_swdge_queues=NQ,
    )
    x_in = nc.dram_tensor("x", [P, TILES * F], fp16, kind="ExternalInput")  # partition-major x^T blocks
    w1_in = nc.dram_tensor("w1", [F, HID], fp16, kind="ExternalInput")
    w2_in = nc.dram_tensor("w2", [HID, NCLS], fp32, kind="ExternalInput")
    b1_in = nc.dram_tensor("b1c", [HID, 1], fp32, kind="ExternalInput")
    b2_in = nc.dram_tensor("b2r", [P, NCLS], fp32, kind="ExternalInput")
    idx1_in = nc.dram_tensor("idx1", [P, ctot1 * 8], mybir.dt.int16, kind="ExternalInput")
    wg1_in = nc.dram_tensor("wg1", [P, ctot1 * S1], fp16, kind="ExternalInput")
    idx2_in = nc.dram_tensor("idx2", [P, ctot2 * 8], mybir.dt.int16, kind="ExternalInput")
    wg2_in = nc.dram_tensor("wg2", [P, ctot2 * S2], fp32, kind="ExternalInput")
    out_d = nc.dram_tensor("out", [TROWS, NCLS], fp32, kind="ExternalOutput")

    xw1_shard = nc.dram_tensor("xw1_shard", [TROWS, HID], fp16)
    xw1_full = nc.dram_tensor("xw1_full", [NCORES * TROWS, HID], fp16, addr_space="Shared")
    hw2_shard = nc.dram_tensor("hw2_shard", [TROWS, NCLS], fp32)
    hw2_full = nc.dram_tensor("hw2_full", [NCORES * TROWS, NCLS], fp32, addr_space="Shared")

    rg = [list(range(NCORES))]
    qctr = [0]

    with tile.TileContext(nc) as tc:
        with (
            tc.tile_pool(name="const", bufs=1) as cpool,
            tc.tile_pool(name="xp", bufs=3) as xp,
            tc.tile_pool(name="xtp", bufs=3) as xtp,
            tc.tile_pool(name="gp", bufs=4) as gp,
            tc.tile_pool(name="gwp", bufs=3) as gwp,
            tc.tile_pool(name="hp", bufs=3) as hp,
            tc.tile_pool(name="ps", bufs=2, space="PSUM") as ps,
            tc.tile_pool(name="ps2", bufs=2, space="PSUM") as ps2,
        ):
            ident = cpool.tile([P, P], fp32)
            make_identity(nc, ident[:])
            w1t = cpool.tile([P, F // P, HID], fp16)  # [128, 4, 64] K-chunks
            nc.sync.dma_start(out=w1t[:], in_=w1_in[:].rearrange("(c p) h -> p c h", p=P))
            w2t = cpool.tile([HID, NCLS], fp32)
            nc.sync.dma_start(out=w2t[:], in_=w2_in[:])
            b1c = cpool.tile([HID, 1], fp32)
            nc.sync.dma_start(out=b1c[:], in_=b1_in[:])
            b2t = cpool.tile([P, NCLS], fp32)
            nc.sync.dma_start(out=b2t[:], in_=b2_in[:])
            # ---- Phase 1: XW1 = x @ W1 per row-tile (x arrives transposed) ----
            for t in range(TILES):
                mm = ps2.tile([P, HID], fp32, space="PSUM", tag="mm1")
                xts = xtp.tile([P, F // P, P], fp16, tag="xts")
                nc.sync.dma_start(
                    out=xts[:],
                    in_=x_in[:, t * F : (t + 1) * F].rearrange("p (c j) -> p c j", j=P),
                )
                for c in range(F // P):
                    nc.tensor.matmul(
                        out=mm[:], lhsT=xts[:, c, :], rhs=w1t[:, c, :],
                        start=(c == 0), stop=(c == F // P - 1),
                    )
                xw1_sb = xp.tile([P, HID], fp16, tag="xw1sb")
                nc.any.tensor_copy(xw1_sb[:], mm[:])
                nc.sync.dma_start(out=xw1_shard[t * P : (t + 1) * P, :], in_=xw1_sb[:])
                for pi, at in enumerate(AG1_AT):
                    if t == at:
                        cb, pr, pb = CORE_BASE1[pi], PIECE_ROWS1[pi], PIECE_BASE1[pi]
                        nc.gpsimd.collective_compute(
                            "AllGather", mybir.AluOpType.bypass, replica_groups=rg,
                            ins=[xw1_shard[cb : cb + pr, :]],
                            outs=[xw1_full[pb : pb + NCORES * pr, :]],
                        )

            # slot tables load during phase 1 / AllGather (needed from phase 3)
            idx1t = cpool.tile([P, ctot1 * 8], mybir.dt.int16)
            nc.sync.dma_start(out=idx1t[:], in_=idx1_in[:])
            wg1t = cpool.tile([P, ctot1 * S1], fp16)
            nc.sync.dma_start(out=wg1t[:], in_=wg1_in[:])
            idx2t = cpool.tile([P, ctot2 * 8], mybir.dt.int16)
            nc.sync.dma_start(out=idx2t[:], in_=idx2_in[:])
            wg2t = cpool.tile([P, ctot2 * S2], fp32)
            nc.sync.dma_start(out=wg2t[:], in_=wg2_in[:])

            # ---- Phases 3/5: aggregation layers ----
            def agg_layer(table_ap, mts, idxt, wgt, gdt, S, width, cnt, stages,
                          off, out_fn, esz, ntok, red_ap_fn=None):
                ci = [0]
                for (t0, t1, cs, c0) in stages:
                    g = gp.tile([P, CAP, esz], gdt, tag="g")
                    # 8 chunks/call: the gather ucode caps at 1024 idxs/call
                    for o in range(0, cs, 8):
                        n = min(8, cs - o)
                        nc.gpsimd.dma_gather(
                            out_ap=g[:, o : o + n, :],
                            in_ap=table_ap[0 : ntok],
                            idxs_ap=idxt[:, (c0 + o) * 8 : (c0 + o + n) * 8],
                            num_idxs=n * P, num_idxs_reg=n * P,
                            elem_size=esz, single_packet=True,
                            queue_num=qctr[0] % NQ,
                        )
                        qctr[0] += 1
                        ci[0] += 1
                    gv = g[:].rearrange("p c (s f) -> p (c s) f", s=S)
                    for t in range(t0, t1):
                        c = int(cnt[t])
                        if c == 0:
                            continue
                        lo = int(off[t]) - c0
                        gw = gwp.tile([P, CAP * S, width], fp32, tag=f"gw{S}")
                        nc.vector.tensor_tensor(
                            out=gw[:, 0 : c * S, :],
                            in0=gv[:, lo * S : (lo + c) * S, :],
                            in1=wgt[:, int(off[t]) * S : (int(off[t]) + c) * S].to_broadcast(
                                [P, c * S, width]
                            ),
                            op=mybir.AluOpType.mult,
                        )
                        if red_ap_fn is None:
                            red = hp.tile([P, width], fp32, tag=f"red{width}")
                            rap = red[:]
                        else:
                            red, rap = None, red_ap_fn(t)
                        nc.vector.tensor_reduce(
                            out=rap, in_=gw[:, 0 : c * S, :].rearrange("p c d -> p d c"),
                            axis=mybir.AxisListType.X, op=mybir.AluOpType.add,
                        )
                        out_fn(t, red)

            # Layer 1 epilogue per tile: ht = relu(agg^T + b1); hw2 = ht^T @ W2
            def l1_out(t, red):
                ht_ps = ps.tile([P, P], fp32, space="PSUM", tag="tp")
                nc.tensor.transpose(out=ht_ps[0:HID, :], in_=red[:], identity=ident[:])
                ht = xtp.tile([HID, P], fp32, tag="ht")
                nc.scalar.activation(
                    ht[:], ht_ps[0:HID, :], mybir.ActivationFunctionType.Relu,
                    bias=b1c[:],
                )
                mm2 = ps2.tile([P, NCLS], fp32, space="PSUM", tag="mm2")
                nc.tensor.matmul(out=mm2[:], lhsT=ht[:], rhs=w2t[:], start=True, stop=True)
                hw2 = hp.tile([P, NCLS], fp32, tag="hw2")
                nc.any.tensor_copy(hw2[:], mm2[:])
                nc.sync.dma_start(out=hw2_shard[t * P : (t + 1) * P, :], in_=hw2[:])
                for pi in ag2_at.get(t, ()):
                    cb, pr, pb = CORE_BASE2[pi], PIECE_ROWS2[pi], PIECE_BASE2[pi]
                    nc.gpsimd.collective_compute(
                        "AllGather", mybir.AluOpType.bypass, replica_groups=rg,
                        ins=[hw2_shard[cb : cb + pr, :]],
                        outs=[hw2_full[pb : pb + NCORES * pr, :]],
                    )

            table1 = xw1_full[:].rearrange("(t s) f -> t (s f)", s=S1)
            agg_layer(table1, mt1, idx1t, wg1t, fp16, S1, HID, cnt1, order1,
                      off1, l1_out, S1 * HID, NCORES * TROWS // S1)

            # ---- Phase 5: layer 2 + batched softmax ----
            logits = cpool.tile([P, TILES, NCLS], fp32)

            table2 = hw2_full[:].rearrange("(t s) f -> t (s f)", s=S2)
            agg_layer(table2, mt2, idx2t, wg2t, fp32, S2, NCLS, cnt2, stages2,
                      off2, lambda t, red: None, S2 * NCLS, NCORES * TROWS // S2,
                      red_ap_fn=lambda t: logits[:, t, :])

            mx = cpool.tile([P, TILES], fp32)
            nc.vector.tensor_reduce(out=mx[:], in_=logits[:], axis=mybir.AxisListType.X, op=mybir.AluOpType.max)
            sh = cpool.tile([P, TILES, NCLS], fp32)
            nc.vector.tensor_tensor(
                out=sh[:], in0=logits[:],
                in1=mx[:].to_broadcast([P, TILES, NCLS]),
                op=mybir.AluOpType.subtract,
            )
            nc.scalar.activation(sh[:], sh[:], mybir.ActivationFunctionType.Exp)
            sm = cpool.tile([P, TILES], fp32)
            nc.vector.tensor_reduce(out=sm[:], in_=sh[:], axis=mybir.AxisListType.X, op=mybir.AluOpType.add)
            nc.vector.reciprocal(sm[:], sm[:])
            nc.vector.tensor_tensor(
                out=sh[:], in0=sh[:],
                in1=sm[:].to_broadcast([P, TILES, NCLS]),
                op=mybir.AluOpType.mult,
            )
            nc.sync.dma_start(
                out=out_d[:].rearrange("(t p) c -> p t c", p=P), in_=sh[:]
            )
    nc.compile()
    return nc


def _prepare(x, src, dst, edge_weight, W1, b1, W2, b2):
    """Build the compiled program + per-core input maps + layout."""
    x = np.asarray(x, dtype=np.float32)
    W1 = np.asarray(W1, dtype=np.float16)
    b1 = np.asarray(b1, dtype=np.float32)
    W2 = np.asarray(W2, dtype=np.float32)
    b2 = np.asarray(b2, dtype=np.float32)

    layout, grids = _preprocess(src, dst, edge_weight)
    owner, lid = layout["owner"], layout["lid"]

    nc = _build(layout)

    assert not np.any(b2), "kernel folds b2 away (spec: zeros)"
    b1cc = b1.reshape(HID, 1).copy()
    b2r = np.broadcast_to(b2, (P, NCLS)).copy()
    in_maps = []
    for r in range(NCORES):
        xr = np.zeros((TROWS, F), dtype=np.float32)
        gl = np.flatnonzero(owner == r)
        xr[lid[gl]] = x[gl]
        # [P, TILES, C, P]: partition-major so each tile DMA reads 2KB/partition
        xr = np.ascontiguousarray(
            xr.reshape(TILES, P, F // P, P).transpose(3, 0, 2, 1)
        ).reshape(P, TILES * F).astype(np.float16)
        ig1, ws1, ig2, ws2 = grids[r]
        in_maps.append(
            {
                "x": xr, "w1": W1, "w2": W2, "b1c": b1cc, "b2r": b2r,
                "idx1": _wrap_idx(ig1), "wg1": ws1.astype(np.float16),
                "idx2": _wrap_idx(ig2), "wg2": ws2,
            }
        )
    return nc, in_maps, layout


def _unshard(shards, layout):
    owner, lid = layout["owner"], layout["lid"]
    out = np.empty((N, NCLS), dtype=np.float32)
    for r in range(NCORES):
        gl = np.flatnonzero(owner == r)
        out[gl] = shards[r][lid[gl]]
    return out


def kernel(x, src, dst, edge_weight, W1, b1, W2, b2):
    global LAST_EXEC_NS
    from concourse import bass_utils

    nc, in_maps, layout = _prepare(x, src, dst, edge_weight, W1, b1, W2, b2)
    res = bass_utils.run_bass_kernel_spmd(
        nc, in_maps, core_ids=list(range(NCORES)), trace=_TRACE
    )
    LAST_EXEC_NS = res.exec_time_ns
    return _unshard([res.results[r]["out"] for r in range(NCORES)], layout)


# revision 22
# speedup vs baseline: 1.0406x; 1.0406x over previous
"""2-layer GCN on 8 Trainium2 NeuronCores (Bass/Tile, SPMD).

softmax(A @ relu(A @ (X@W1) + b1) @ W2 + b2), N=50k nodes, E=800k edges.

Strategy (1D graph partition, token-packed gathers, 4 SWDGE queues):
- Nodes sharded 6250/core by global in-degree rank round-robin (rank k ->
  core k%8, local id k//8), padded to 6272 = 49*128 table rows per core.
  Local ids are descending-degree, so 128-node dst tiles have near-uniform
  degree and fixed per-tile slot counts waste little padding.
- Edges partitioned by dst owner. Layer-1 XW1 table is fp16 [50176, 64]
  -> 25088 pair-tokens of 256B; layer-2 HW2 stays fp32 [50176, 16] ->
  12544 4-row tokens of 256B (fp16 there would blow the softmax error
  budget). Token counts fit int16 gather indices.
- Per dst node, in-edges are deduped by (dst, token) into slots; each slot
  gathers one 256B token and a per-sub-row weight grid selects + weights
  rows during the DVE multiply (fp32 accumulate); a chunk-sum reduce
  completes the segment sum.
- dma_gather calls (1024 idxs each) round-robin over 4 SWDGE queues; the
  SWDGE queue descriptor rate (~11ns/desc/queue) is the wall.
- Slots are token-sorted per node, so each gather call's in_ap is sliced
  to [0, call_max_token]: early calls only depend on the first AllGather
  pieces and start while later pieces are still in flight.
- XW1 / HW2 tables are exchanged with 4 piece-wise AllGather collectives
  (piece-core-major table layout) overlapped with phase-1 / layer-1 compute.
"""

import sys

sys.path.insert(0, "/opt/trn_rl_repo")

import numpy as np

N = 50000
E = 800000
F = 512
HID = 64
NCLS = 16
NCORES = 8
P = 128
NPC = N // NCORES  # 6250
TILES = 49
TROWS = TILES * P  # 6272
CAP = 40  # max chunks per gather stage
NQ = 4  # SWDGE queues
SH1, SH2 = 1, 2  # token shifts: L1 fp16 pairs (256B), L2 fp32 4-row (256B)
S1, S2 = 1 << SH1, 1 << SH2
# AG1 is a single whole-table collective issued right after the (fast fp16)
# phase-1 finishes: one big transfer beats a piece chain's per-op dispatch.
# AG2 is split so one piece covers most tiles and issues mid-stream (one
# ~10us gather-queue stall) and two small pieces trail the stream.
PIECE_T1 = [(0, TILES)]


def _piece_layout(piece_t):
    rows = [(t1 - t0) * P for (t0, t1) in piece_t]  # rows per core per piece
    cbs = [t0 * P for (t0, t1) in piece_t]          # row offset within a core
    pbs = [0]                                       # row offset in full table
    for pr in rows[:-1]:
        pbs.append(pbs[-1] + NCORES * pr)
    return rows, cbs, pbs


PIECE_ROWS1, CORE_BASE1, PIECE_BASE1 = _piece_layout(PIECE_T1)
AG1_AT = [TILES - 1]

_TRACE = False
LAST_EXEC_NS = None


def _build_grid(es_row, dl, ew, shift):
    """Slot grid for one core+layer: tokens = row>>shift, dedup (dst, token).

    Returns ig [P, ctot] int16 token grid, wsub [P, ctot*S] f32 sub-weights,
    cnt[TILES] per-tile chunk counts, stages list, off[TILES] col offsets.
    """
    S = 1 << shift
    tok = es_row >> shift
    sub = es_row & (S - 1)
    ntok_max = int(tok.max()) + 1 if len(tok) else 1
    order = np.lexsort((tok, dl))
    dls, toks, subs, ews = dl[order], tok[order], sub[order], ew[order]
    key = dls * ntok_max + toks
    new = np.r_[True, np.diff(key) != 0]
    slot_id = np.cumsum(new) - 1  # per (sorted) edge -> slot
    slot_dl = dls[new]
    slot_tok = toks[new]
    nslots = len(slot_tok)

    kd = np.bincount(slot_dl, minlength=NPC)  # unique-token count per node
    kd_pad = np.concatenate([kd, np.zeros(TROWS - NPC, np.int64)])
    cnt = kd_pad.reshape(TILES, P).max(1)  # per-tile chunk count

    # stages: consecutive tiles while chunk sum <= CAP
    stages = []
    off = np.zeros(TILES, dtype=np.int64)
    t0 = 0
    ctot = 0
    while t0 < TILES:
        t1, cs = t0, 0
        while t1 < TILES and cs + cnt[t1] <= CAP:
            off[t1] = ctot + cs
            cs += cnt[t1]
            t1 += 1
        stages.append((t0, t1, int(cs), ctot))
        ctot += cs
        t0 = t1
    ctot = int(ctot)

    # slot columns: per node, j-th slot at off[tile] + j
    starts = np.r_[0, np.cumsum(kd)[:-1]]
    j = np.arange(nslots) - starts[slot_dl]
    tl = slot_dl // P
    prow = slot_dl % P
    col = off[tl] + j

    ig = np.zeros((P, ctot), dtype=np.int16)
    ig[prow, col] = slot_tok.astype(np.int16)
    wsub = np.zeros((P, ctot * S), dtype=np.float32)
    e_col = col[slot_id]  # per sorted edge
    e_prow = prow[slot_id]
    np.add.at(wsub, (e_prow, e_col * S + subs), ews)
    return ig, wsub, cnt, stages, off, ctot


def _preprocess(src, dst, edge_weight):
    src = np.asarray(src).astype(np.int64).ravel()
    dst = np.asarray(dst).astype(np.int64).ravel()
    w = np.asarray(edge_weight).astype(np.float32).ravel()

    tdeg = np.bincount(dst, minlength=N)
    grank = np.empty(N, dtype=np.int64)
    grank[np.argsort(-tdeg, kind="stable")] = np.arange(N)
    owner_of = grank % NCORES
    lid_of = grank // NCORES  # descending-degree local order

    # full tables are piece-major then core-major (AllGather piece layout);
    # each layer has its own piece set, hence its own row numbering
    def row_map(cbs, rows, pbs):
        piece_of = np.searchsorted(np.array(cbs + [TROWS]), lid_of, side="right") - 1
        pb = np.array(pbs)[piece_of]
        prw = np.array(rows)[piece_of]
        cb = np.array(cbs)[piece_of]
        return pb + owner_of * prw + (lid_of - cb)

    row1_of = row_map(CORE_BASE1, PIECE_ROWS1, PIECE_BASE1)

    owner_dst = owner_of[dst]
    per_core = []
    for r in range(NCORES):
        m = owner_dst == r
        per_core.append((src[m], dst[m], w[m]))
    cores = []
    for es, ed, ew in per_core:
        dl = lid_of[ed]
        g1 = _build_grid(row1_of[es], dl, ew, SH1)  # layer 1: fp16 pair tokens
        cores.append([g1])

    # unify stage/offset structure across cores (max per-tile counts) so one
    # compiled program fits all cores
    def unify(idx, last_alone=False):
        cnt = np.stack([c[idx][2] for c in cores]).max(0)
        stages = []
        off = np.zeros(TILES, dtype=np.int64)
        lim = TILES - 1 if last_alone else TILES
        t0 = 0
        ctot = 0
        while t0 < lim:
            t1, cs = t0, 0
            while t1 < lim and cs + cnt[t1] <= CAP:
                off[t1] = ctot + cs
                cs += cnt[t1]
                t1 += 1
            stages.append((t0, t1, int(cs), ctot))
            ctot += cs
            t0 = t1
        if last_alone:
            off[TILES - 1] = ctot
            stages.append((TILES - 1, TILES, int(cnt[TILES - 1]), ctot))
            ctot += cnt[TILES - 1]
        return cnt, stages, off, int(ctot)

    # layer 1's last stage is tile 48 alone: AG2's final (single-tile) piece
    # only waits for one small tile's reduce, not a whole trailing stage
    uni1 = unify(0, last_alone=True)

    # Layer-1 stages are processed as [a..48) stages, then the tile-48
    # stage, then [0..a) stages. AG2 piece (a,49) issues right after tile
    # 48 (one mid-stream queue stall); pieces (0,c) and (c,a) issue at the
    # last two processed stages so only ~2 small transfers trail the
    # stream.
    stages1 = uni1[1]
    sbounds = [st[0] for st in stages1]
    a = min((t for t in sbounds if 0 < t < TILES - 1), key=lambda t: abs(t - 15))
    grp2 = [st for st in stages1 if st[0] < a]
    c = grp2[-1][0]
    piece_t2 = [(0, c), (c, a), (a, TILES)]
    rows2, cbs2, pbs2 = _piece_layout(piece_t2)
    order1 = ([st for st in stages1 if a <= st[0] < TILES - 1]
              + [st for st in stages1 if st[0] == TILES - 1]
              + grp2)
    ag2_at = {grp2[-2][1] - 1: [0], grp2[-1][1] - 1: [1], TILES - 1: [2]}

    row2_of = row_map(cbs2, rows2, pbs2)
    for r, (es, ed, ew) in enumerate(per_core):
        dl = lid_of[ed]
        cores[r].append(_build_grid(row2_of[es], dl, ew, SH2))  # layer 2: fp32 4-row tokens
    uni2 = unify(1)

    # re-grid each core onto the unified layout
    def regrid(r, idx, uni, shift):
        S = 1 << shift
        ig_c, wsub_c, cnt_c, stages_c, off_c, ctot_c = cores[r][idx]
        cnt_u, stages_u, off_u, ctot_u = uni
        ig = np.zeros((P, ctot_u), dtype=np.int16)
        wsub = np.zeros((P, ctot_u * S), dtype=np.float32)
        for t in range(TILES):
            c = int(cnt_c[t])
            if c == 0:
                continue
            src_lo = int(off_c[t])
            dst_lo = int(off_u[t])
            ig[:, dst_lo:dst_lo + c] = ig_c[:, src_lo:src_lo + c]
            wsub[:, dst_lo * S:(dst_lo + c) * S] = wsub_c[:, src_lo * S:(src_lo + c) * S]
        return ig, wsub

    grids = []
    for r in range(NCORES):
        ig1, ws1 = regrid(r, 0, uni1, SH1)
        ig2, ws2 = regrid(r, 1, uni2, SH2)
        grids.append((ig1, ws1, ig2, ws2))

    # per-gather-call max token (over all cores): slots are token-sorted per
    # node, so early calls touch only early AllGather pieces -- slicing the
    # gather in_ap to [0, maxtok] lets them start before the later pieces
    # land.
    def call_maxtoks(gidx, uni):
        _, stages, _, _ = uni
        igs = np.stack([g[gidx] for g in grids])  # [NCORES, P, ctot]
        mts = []
        for (t0, t1, cs, c0) in stages:
            for o in range(0, cs, 8):
                n = min(8, cs - o)
                mts.append(int(igs[:, :, c0 + o : c0 + o + n].max()))
        return mts

    layout = dict(uni1=uni1, uni2=uni2, owner=owner_of, lid=lid_of,
                  mt1=call_maxtoks(0, uni1), mt2=call_maxtoks(2, uni2),
                  order1=order1, ag2_at=ag2_at,
                  ag2=(rows2, cbs2, pbs2))
    return layout, grids


def _wrap_idx(ig):
    """[128, C] token grid -> dma_gather wrapped idx array [128, C*8] int16."""
    seq = ig.T.reshape(-1)  # position q = c*128 + p
    cols = seq.shape[0] // 16
    seqm = seq.reshape(cols, 16).T  # [16, cols]
    return np.tile(seqm, (8, 1)).astype(np.int16)  # [128, cols]


def _build(layout):
    import concourse.bacc as bacc
    import concourse.tile as tile
    import concourse.mybir as mybir
    from concourse.masks import make_identity

    cnt1, stages1, off1, ctot1 = layout["uni1"]
    cnt2, stages2, off2, ctot2 = layout["uni2"]
    mt1, mt2 = layout["mt1"], layout["mt2"]
    order1, ag2_at = layout["order1"], layout["ag2_at"]
    PIECE_ROWS2, CORE_BASE2, PIECE_BASE2 = layout["ag2"]
    fp32 = mybir.dt.float32
    fp16 = mybir.dt.float16

    nc = bacc.Bacc(
        "TRN2", target_bir_lowering=False, debug=False, num_devices=NCORES,
        num_swdge_queues=NQ,
    )
    x_in = nc.dram_tensor("x", [P, TILES * F], fp16, kind="ExternalInput")  # partition-major x^T blocks
    w1_in = nc.dram_tensor("w1", [F, HID], fp16, kind="ExternalInput")
    w2_in = nc.dram_tensor("w2", [HID, NCLS], fp32, kind="ExternalInput")
    b1_in = nc.dram_tensor("b1c", [HID, 1], fp32, kind="ExternalInput")
    b2_in = nc.dram_tensor("b2r", [P, NCLS], fp32, kind="ExternalInput")
    idx1_in = nc.dram_tensor("idx1", [P, ctot1 * 8], mybir.dt.int16, kind="ExternalInput")
    wg1_in = nc.dram_tensor("wg1", [P, ctot1 * S1], fp16, kind="ExternalInput")
    idx2_in = nc.dram_tensor("idx2", [P, ctot2 * 8], mybir.dt.int16, kind="ExternalInput")
    wg2_in = nc.dram_tensor("wg2", [P, ctot2 * S2], fp32, kind="ExternalInput")
    out_d = nc.dram_tensor("out", [TROWS, NCLS], fp32, kind="ExternalOutput")

    xw1_shard = nc.dram_tensor("xw1_shard", [TROWS, HID], fp16)
    xw1_full = nc.dram_tensor("xw1_full", [NCORES * TROWS, HID], fp16, addr_space="Shared")
    hw2_shard = nc.dram_tensor("hw2_shard", [TROWS, NCLS], fp32)
    hw2_full = nc.dram_tensor("hw2_full", [NCORES * TROWS, NCLS], fp32, addr_space="Shared")

    rg = [list(range(NCORES))]
    qctr = [0]

    with tile.TileContext(nc) as tc:
        with (
            tc.tile_pool(name="const", bufs=1) as cpool,
            tc.tile_pool(name="xp", bufs=3) as xp,
            tc.tile_pool(name="xtp", bufs=3) as xtp,
            tc.tile_pool(name="gp", bufs=4) as gp,
            tc.tile_pool(name="gwp", bufs=3) as gwp,
            tc.tile_pool(name="hp", bufs=3) as hp,
            tc.tile_pool(name="ps", bufs=2, space="PSUM") as ps,
            tc.tile_pool(name="ps2", bufs=2, space="PSUM") as ps2,
        ):
            ident = cpool.tile([P, P], fp32)
            make_identity(nc, ident[:])
            w1t = cpool.tile([P, F // P, HID], fp16)  # [128, 4, 64] K-chunks
            nc.sync.dma_start(out=w1t[:], in_=w1_in[:].rearrange("(c p) h -> p c h", p=P))
            w2t = cpool.tile([HID, NCLS], fp32)
            nc.sync.dma_start(out=w2t[:], in_=w2_in[:])
            b1c = cpool.tile([HID, 1], fp32)
            nc.sync.dma_start(out=b1c[:], in_=b1_in[:])
            b2t = cpool.tile([P, NCLS], fp32)
            nc.sync.dma_start(out=b2t[:], in_=b2_in[:])
            # ---- Phase 1: XW1 = x @ W1 per row-tile (x arrives transposed) ----
            for t in range(TILES):
                mm = ps2.tile([P, HID], fp32, space="PSUM", tag="mm1")
                xts = xtp.tile([P, F // P, P], fp16, tag="xts")
                nc.sync.dma_start(
                    out=xts[:],
                    in_=x_in[:, t * F : (t + 1) * F].rearrange("p (c j) -> p c j", j=P),
                )
                for c in range(F // P):
                    nc.tensor.matmul(
                        out=mm[:], lhsT=xts[:, c, :], rhs=w1t[:, c, :],
                        start=(c == 0), stop=(c == F // P - 1),
                    )
                xw1_sb = xp.tile([P, HID], fp16, tag="xw1sb")
                nc.any.tensor_copy(xw1_sb[:], mm[:])
                nc.sync.dma_start(out=xw1_shard[t * P : (t + 1) * P, :], in_=xw1_sb[:])
                for pi, at in enumerate(AG1_AT):
                    if t == at:
                        cb, pr, pb = CORE_BASE1[pi], PIECE_ROWS1[pi], PIECE_BASE1[pi]
                        nc.gpsimd.collective_compute(
                            "AllGather", mybir.AluOpType.bypass, replica_groups=rg,
                            ins=[xw1_shard[cb : cb + pr, :]],
                            outs=[xw1_full[pb : pb + NCORES * pr, :]],
                        )

            # slot tables load during phase 1 / AllGather (needed from phase 3)
            idx1t = cpool.tile([P, ctot1 * 8], mybir.dt.int16)
            nc.sync.dma_start(out=idx1t[:], in_=idx1_in[:])
            wg1t = cpool.tile([P, ctot1 * S1], fp16)
            nc.sync.dma_start(out=wg1t[:], in_=wg1_in[:])
            idx2t = cpool.tile([P, ctot2 * 8], mybir.dt.int16)
            nc.sync.dma_start(out=idx2t[:], in_=idx2_in[:])
            wg2t = cpool.tile([P, ctot2 * S2], fp32)
            nc.sync.dma_start(out=wg2t[:], in_=wg2_in[:])

            # ---- Phases 3/5: aggregation layers ----
            def agg_layer(table_ap, mts, idxt, wgt, gdt, S, width, cnt, stages,
                          off, out_fn, esz, ntok, red_ap_fn=None):
                ci = [0]
                for (t0, t1, cs, c0) in stages:
                    g = gp.tile([P, CAP, esz], gdt, tag="g")
                    # 8 chunks/call: the gather ucode caps at 1024 idxs/call
                    for o in range(0, cs, 8):
                        n = min(8, cs - o)
                        nc.gpsimd.dma_gather(
                            out_ap=g[:, o : o + n, :],
                            in_ap=table_ap[0 : ntok],
                            idxs_ap=idxt[:, (c0 + o) * 8 : (c0 + o + n) * 8],
                            num_idxs=n * P, num_idxs_reg=n * P,
                            elem_size=esz, single_packet=True,
                            queue_num=qctr[0] % NQ,
                        )
                        qctr[0] += 1
                        ci[0] += 1
                    gv = g[:].rearrange("p c (s f) -> p (c s) f", s=S)
                    for t in range(t0, t1):
                        c = int(cnt[t])
                        if c == 0:
                            continue
                        lo = int(off[t]) - c0
                        gw = gwp.tile([P, CAP * S, width], fp32, tag=f"gw{S}")
                        nc.vector.tensor_tensor(
                            out=gw[:, 0 : c * S, :],
                            in0=gv[:, lo * S : (lo + c) * S, :],
                            in1=wgt[:, int(off[t]) * S : (int(off[t]) + c) * S].to_broadcast(
                                [P, c * S, width]
                            ),
                            op=mybir.AluOpType.mult,
                        )
                        if red_ap_fn is None:
                            red = hp.tile([P, width], fp32, tag=f"red{width}")
                            rap = red[:]
                        else:
                            red, rap = None, red_ap_fn(t)
                        nc.vector.tensor_reduce(
                            out=rap, in_=gw[:, 0 : c * S, :].rearrange("p c d -> p d c"),
                            axis=mybir.AxisListType.X, op=mybir.AluOpType.add,
                        )
                        out_fn(t, red)

            # Layer 1 epilogue per tile: ht = relu(agg^T + b1); hw2 = ht^T @ W2
            def l1_out(t, red):
                ht_ps = ps.tile([P, P], fp32, space="PSUM", tag="tp")
                nc.tensor.transpose(out=ht_ps[0:HID, :], in_=red[:], identity=ident[:])
                ht = xtp.tile([HID, P], fp32, tag="ht")
                nc.scalar.activation(
                    ht[:], ht_ps[0:HID, :], mybir.ActivationFunctionType.Relu,
                    bias=b1c[:],
                )
                mm2 = ps2.tile([P, NCLS], fp32, space="PSUM", tag="mm2")
                nc.tensor.matmul(out=mm2[:], lhsT=ht[:], rhs=w2t[:], start=True, stop=True)
                hw2 = hp.tile([P, NCLS], fp32, tag="hw2")
                nc.any.tensor_copy(hw2[:], mm2[:])
                nc.sync.dma_start(out=hw2_shard[t * P : (t + 1) * P, :], in_=hw2[:])
                for pi in ag2_at.get(t, ()):
                    cb, pr, pb = CORE_BASE2[pi], PIECE_ROWS2[pi], PIECE_BASE2[pi]
                    nc.gpsimd.collective_compute(
                        "AllGather", mybir.AluOpType.bypass, replica_groups=rg,
                        ins=[hw2_shard[cb : cb + pr, :]],
                        outs=[hw2_full[pb : pb + NCORES * pr, :]],
                    )

            table1 = xw1_full[:].rearrange("(t s) f -> t (s f)", s=S1)
            agg_layer(table1, mt1, idx1t, wg1t, fp16, S1, HID, cnt1, order1,
                      off1, l1_out, S1 * HID, NCORES * TROWS // S1)

            # ---- Phase 5: layer 2 + batched softmax ----
            logits = cpool.tile([P, TILES, NCLS], fp32)

            table2 = hw2_full[:].rearrange("(t s) f -> t (s f)", s=S2)
            agg_layer(table2, mt2, idx2t, wg2t, fp32, S2, NCLS, cnt2, stages2,
                      off2, lambda t, red: None, S2 * NCLS, NCORES * TROWS // S2,
                      red_ap_fn=lambda t: logits[:, t, :])

            mx = cpool.tile([P, TILES], fp32)
            nc.vector.tensor_reduce(out=mx[:], in_=logits[:], axis=mybir.AxisListType.X, op=mybir.AluOpType.max)
            sh = cpool.tile([P, TILES, NCLS], fp32)
            nc.vector.tensor_tensor(
                out=sh[:], in0=logits[:],
                in1=mx[:].to_broadcast([P, TILES, NCLS]),
                op=mybir.AluOpType.subtract,
            )
            nc.scalar.activation(sh[:], sh[:], mybir.ActivationFunctionType.Exp)
            sm = cpool.tile([P, TILES], fp32)
            nc.vector.tensor_reduce(out=sm[:], in_=sh[:], axis=mybir.AxisListType.X, op=mybir.AluOpType.add)
            nc.vector.reciprocal(sm[:], sm[:])
            nc.vector.tensor_tensor(
                out=sh[:], in0=sh[:],
                in1=sm[:].to_broadcast([P, TILES, NCLS]),
                op=mybir.AluOpType.mult,
            )
            nc.sync.dma_start(
                out=out_d[:].rearrange("(t p) c -> p t c", p=P), in_=sh[:]
            )
    nc.compile()
    return nc


def _prepare(x, src, dst, edge_weight, W1, b1, W2, b2):
    """Build the compiled program + per-core input maps + layout."""
    x = np.asarray(x, dtype=np.float32)
    W1 = np.asarray(W1, dtype=np.float16)
    b1 = np.asarray(b1, dtype=np.float32)
    W2 = np.asarray(W2, dtype=np.float32)
    b2 = np.asarray(b2, dtype=np.float32)

    layout, grids = _preprocess(src, dst, edge_weight)
    owner, lid = layout["owner"], layout["lid"]

    nc = _build(layout)

    assert not np.any(b2), "kernel folds b2 away (spec: zeros)"
    b1cc = b1.reshape(HID, 1).copy()
    b2r = np.broadcast_to(b2, (P, NCLS)).copy()
    in_maps = []
    for r in range(NCORES):
        xr = np.zeros((TROWS, F), dtype=np.float32)
        gl = np.flatnonzero(owner == r)
        xr[lid[gl]] = x[gl]
        # [P, TILES, C, P]: partition-major so each tile DMA reads 2KB/partition
        xr = np.ascontiguousarray(
            xr.reshape(TILES, P, F // P, P).transpose(3, 0, 2, 1)
        ).reshape(P, TILES * F).astype(np.float16)
        ig1, ws1, ig2, ws2 = grids[r]
        in_maps.append(
            {
                "x": xr, "w1": W1, "w2": W2, "b1c": b1cc, "b2r": b2r,
                "idx1": _wrap_idx(ig1), "wg1": ws1.astype(np.float16),
                "idx2": _wrap_idx(ig2), "wg2": ws2,
            }
        )
    return nc, in_maps, layout


def _unshard(shards, layout):
    owner, lid = layout["owner"], layout["lid"]
    out = np.empty((N, NCLS), dtype=np.float32)
    for r in range(NCORES):
        gl = np.flatnonzero(owner == r)
        out[gl] = shards[r][lid[gl]]
    return out


def kernel(x, src, dst, edge_weight, W1, b1, W2, b2):
    global LAST_EXEC_NS
    from concourse import bass_utils

    nc, in_maps, layout = _prepare(x, src, dst, edge_weight, W1, b1, W2, b2)
    res = bass_utils.run_bass_kernel_spmd(
        nc, in_maps, core_ids=list(range(NCORES)), trace=_TRACE
    )
    LAST_EXEC_NS = res.exec_time_ns
    return _unshard([res.results[r]["out"] for r in range(NCORES)], layout)
